# revision 48
# baseline (speedup 1.0000x reference)
"""Trainium2 Bass kernel for nn_Attention_54391465836966.

The reference's .reshape calls are RAW byte reinterpretations: token matrix
T = content_feat[b] bytes viewed [S, C] (not a transpose), and s (token-major
[S, C]) is viewed [C, S] before the 1x1 conv.  The host passes every input
pre-arranged into its exact SBUF image (one [128, X] contiguous DMA each, in
bf16), with the token views pre-transposed to channel-major, so the device
does no PE transposes; the s view is realized with SBUF->SBUF DMAs that
re-pair token rows (s2d[r] = tokens (2r, 2r+1) concatenated).

Per core (b = core//4, n = core%4), channel-major [C, S] throughout:
  ctok = cfT + posT ; ctmp = compT + posT
  qT = Wq^T ctok ; kT = Wkv[:, :C]^T ctmp ; v = ctmp^T Wkv[:, C:]
  per head h: P = exp(scale k_h^T q); o_h = (v_h^T P) / Z   (Z via ones col)
  s_tok = packed^T Wproj                                     (token-major)
  const (token-quarter n, full scale): s_cq = ctokQ^T Wproj + bproj
  out_p = WconvT[:C]^T s2d + WconvT[quarter]^T s2d_cq + bconv/4
  out_cf = WconvT[C:, out-quarter]^T cf_raw                  (host-placed)
Host sums the 4 component partials per batch and places out_cf quarter rows.
The affine const terms are distributed so no gated-zero work exists.

Dtypes: bf16 throughout (PE rate = fp32r, half the DMA/SBUF traffic; DVE
adds get the 2x mode); PSUM and the softmax-normalization scratch stay f32.

Schedule: attention is ACT(exp)-bound at ~1.2us/kt, so only v and the
(kT, qT) pair for head pair 0 are computed up front; everything else that
does not gate the exp stream — the remaining k/q groups, the const paths,
and the per-head-pair proj partial sums — is emitted INTO the head loop to
fill PE slack under the exps.  Only the last pair's proj round, the s2d
re-pair, and the conv remain in the tail.  The z-scratch is double-buffered
by head parity; head 6 (cheap even-parity norm) is processed last.
"""
import sys

sys.path.insert(0, "/opt/trn_rl_repo")

import numpy as np

N_CORES = 8
B, C, H, W = 2, 512, 32, 32
S = H * W  # 1024
NH, HD = 8, 64
SCALE = HD ** -0.5

_CACHE = {}


def _img(x, cols):
    """[512, cols] matrix -> its [128, 4*cols] SBUF image (4 row-blocks
    side by side), in bf16."""
    import ml_dtypes
    return np.ascontiguousarray(
        x.reshape(4, 128, cols).transpose(1, 0, 2).reshape(128, 4 * cols)
    ).astype(ml_dtypes.bfloat16)


def _build():
    if "nc" in _CACHE:
        return _CACHE["nc"]
    from contextlib import ExitStack

    import concourse.bacc as bacc
    import concourse.mybir as mybir
    import concourse.tile as tile

    f32 = mybir.dt.float32
    bf16 = mybir.dt.bfloat16
    EXP = mybir.ActivationFunctionType.Exp

    nc = bacc.Bacc("TRN2", target_bir_lowering=False, debug=False,
                   num_devices=N_CORES)

    din = lambda n, s: nc.dram_tensor(n, s, mybir.dt.bfloat16,
                                      kind="ExternalInput").ap()
    ctm_d = din("ctm", [128, 4096])      # (compT + posT) image
    ctk_d = din("ctk", [128, 4096])      # (cfT + posT) image
    cfr_d = din("cfr", [128, 4096])      # raw content_feat[b] image
    wkq_d = din("wkq", [128, 4096])      # [Wkv[:, :C] | Wq] images
    wv_d = din("wv", [128, 2048])        # Wkv[:, C:] image
    wproj_d = din("wproj", [128, 2048])  # Wproj image
    wcvs_d = din("wcvs", [128, 2048])    # WconvT[:C] image
    wcvcq_d = din("wcvcq", [128, 512])   # WconvT[C:, out-quarter] image
    wcvsq_d = din("wcvsq", [128, 512])   # WconvT[128n:128(n+1), :]
    ctkq_d = din("ctkq", [128, 1024])    # ctok[:, token-quarter] image
    bias_d = din("bias2", [1, 1024])     # [bproj, bconv/4]
    out_p = nc.dram_tensor("out_p", [C, S], bf16, kind="ExternalOutput").ap()
    out_cf = nc.dram_tensor("out_cf", [128, S], bf16,
                            kind="ExternalOutput").ap()

    with tile.TileContext(nc) as tc, ExitStack() as ctx:
        main = ctx.enter_context(tc.tile_pool(name="main", bufs=1))

        ones = main.tile([1, 512], bf16, tag="ones")
        nc.gpsimd.memset(ones[:], 1.0)

        # ---- front-critical DMAs, split across the SP and Pool queues so
        # issue overhead (~1.5us/DMA on SP) parallelizes ----
        ctm_big = main.tile([128, 4096], bf16, tag="ctmB")
        ctk_big = main.tile([128, 4096], bf16, tag="ctkB")
        ctmp = [ctm_big[:, 1024 * j:1024 * (j + 1)] for j in range(4)]
        ctok = [ctk_big[:, 1024 * j:1024 * (j + 1)] for j in range(4)]
        wv_sb = main.tile([128, 2048], bf16, tag="wv")
        wkq_big = main.tile([128, 4096], bf16, tag="wkqB")
        wk_sb = wkq_big[:, 0:2048]
        wq_sb = wkq_big[:, 2048:4096]
        nc.sync.dma_start(ctm_big[:], ctm_d[:])
        nc.gpsimd.dma_start(ctk_big[:], ctk_d[:])
        nc.sync.dma_start(wkq_big[:], wkq_d[:])
        nc.gpsimd.dma_start(wv_sb[:], wv_d[:])

        # ---- late weights / const-path inputs (Pool queue) ----
        cfr_big = main.tile([128, 4096], bf16, tag="cfrB")
        cfr_sb = [cfr_big[:, 1024 * j:1024 * (j + 1)] for j in range(4)]
        wproj_sb = main.tile([128, 2048], bf16, tag="wp")
        wcvs_sb = main.tile([128, 2048], bf16, tag="wcs")
        wcvcq_sb = main.tile([128, 512], bf16, tag="wcc")
        wcvsq_sb = main.tile([128, 512], bf16, tag="wcsq")
        ctokq = main.tile([128, 1024], bf16, tag="ctokq")
        bias_sb = main.tile([1, 1024], bf16, tag="bias")
        def late_dmas():
            nc.gpsimd.dma_start(wproj_sb[:], wproj_d[:])
            nc.gpsimd.dma_start(cfr_big[:], cfr_d[:])
            nc.gpsimd.dma_start(wcvcq_sb[:], wcvcq_d[:])
            nc.gpsimd.dma_start(ctokq[:], ctkq_d[:])
            nc.gpsimd.dma_start(bias_sb[:], bias_d[:])
            nc.gpsimd.dma_start(wcvs_sb[:], wcvs_d[:])
            nc.gpsimd.dma_start(wcvsq_sb[:], wcvsq_d[:])

        # norm scratch (x2 by parity, shared zs2) + late tiles
        zraw = [main.tile([1, S], f32, tag=f"zraw{i}", name=f"zraw{i}")
                for i in range(2)]
        zs2 = main.tile([1, S], f32, tag="zs2")
        zinv = [main.tile([1, S], f32, tag=f"zinv{i}", name=f"zinv{i}")
                for i in range(2)]
        zbc = [main.tile([128, S], f32, tag=f"zbc{i}", name=f"zbc{i}")
               for i in range(2)]
        ocf_sb = main.tile([128, S], bf16, tag="ctkB", name="ocf")
        packed = [main.tile([128, S], bf16, tag=f"pk{j}", name=f"pk{j}")
                  for j in range(4)]
        outp = [main.tile([128, S], bf16, tag=f"op{j}", name=f"op{j}")
                for j in range(4)]
        kT = [main.tile([128, S], bf16, tag=f"kT{j}", name=f"kT{j}")
              for j in range(4)]
        qT = [main.tile([128, S], bf16, tag=f"qT{j}", name=f"qT{j}")
              for j in range(4)]
        # v pairs: [p, (i, 128h + [64 d | Z | 63 pad])], pads/Z = 1.0
        v_sb = [main.tile([128, 2048], bf16, tag=f"v{t}", name=f"v{t}")
                for t in range(4)]
        # proj partial accumulators (f32) ride the dead pos slot; final s
        # tiles ride cmp; the re-paired s2d view rides cfr
        s_acc_big = main.tile([128, 4096], f32, tag="ctmB", name="sacc")
        s_acc = [s_acc_big[:, 512 * i:512 * (i + 1)] for i in range(8)]
        s_sb_big = main.tile([128, 4096], bf16, tag="wkqB", name="ssb")
        s_sb = [s_sb_big[:, 512 * i:512 * (i + 1)] for i in range(8)]
        s2d_big = main.tile([128, 4096], bf16, tag="cfrB", name="s2db")
        s2d = [s2d_big[:, 1024 * i:1024 * (i + 1)] for i in range(4)]

        def kq_part(psum, j, part):
            """One of four (kT/qT, qc) projection groups for head-pair j."""
            kind, qc = part % 2, part // 2
            w, act, dst = ((wk_sb, ctmp, kT) if kind == 0
                           else (wq_sb, ctok, qT))
            acc = psum.tile([128, 512], f32, tag="mm")
            for k in range(4):
                nc.tensor.matmul(
                    acc[:],
                    w[:, 512 * k + 128 * j:512 * k + 128 * (j + 1)],
                    act[k][:, 512 * qc:512 * (qc + 1)],
                    start=(k == 0), stop=(k == 3))
            nc.vector.tensor_copy(dst[j][:, 512 * qc:512 * (qc + 1)], acc[:])

        def v_group(psum, kt):
            acc = psum.tile([128, 512], f32, tag="mm")
            for k in range(4):
                nc.tensor.matmul(acc[:],
                                 ctmp[k][:, 128 * kt:128 * (kt + 1)],
                                 wv_sb[:, 512 * k:512 * (k + 1)],
                                 start=(k == 0), stop=(k == 3))
            dst = v_sb[kt // 2][:, 1024 * (kt % 2):1024 * (kt % 2) + 1024]
            nc.scalar.copy(
                dst.rearrange("p (m c) -> p m c", m=8)[:, :, 0:64],
                acc[:].rearrange("p (m c) -> p m c", m=8))

        with tc.tile_pool(name="psA", bufs=2, space="PSUM") as ps:
            for t in range(4):
                nc.gpsimd.memset(v_sb[t][:], 1.0)
            for part in range(4):
                kq_part(ps, 0, part)
            for kt in range(2):
                v_group(ps, kt)

        # ---- attention, with remaining work streamed into PE slack ----
        with tc.tile_pool(name="psT", bufs=2, space="PSUM") as psT:
            attn_ctx = ExitStack()
            psS = attn_ctx.enter_context(
                tc.tile_pool(name="psS", bufs=2, space="PSUM"))
            psO = attn_ctx.enter_context(
                tc.tile_pool(name="psO", bufs=1, space="PSUM"))
            ptp = [main.tile([128, 2048], bf16, tag=f"pt{t}", name=f"pt{t}")
                   for t in range(4)]
            s_cq = [main.tile([128, 512], bf16, tag=f"scq{i}", name=f"scq{i}")
                    for i in range(2)]
            s2d_cq = main.tile([128, S], bf16, tag="s2dcq")

            def scq_part(i):
                acc = psT.tile([128, 512], f32, tag="mm")
                nc.tensor.matmul(acc[:], ones[0:1, 0:128],
                                 bias_sb[0:1, 0:512], start=True, stop=False)
                for a in range(4):
                    nc.tensor.matmul(
                        acc[:],
                        ctokq[:, 256 * a + 128 * i:256 * a + 128 * (i + 1)],
                        wproj_sb[:, 512 * a:512 * (a + 1)],
                        start=False, stop=(a == 3))
                nc.vector.tensor_copy(s_cq[i][:], acc[:])
                if i == 1:
                    for g in range(2):
                        for sh in range(2):
                            eng = nc.sync if g == 0 else nc.gpsimd
                            eng.dma_start(
                                s2d_cq[64 * sh:64 * sh + 64,
                                       512 * g:512 * g + 512],
                                s_cq[sh][g:128:2, :])

            def ocf_part(half):
                acc = psT.tile([128, 512], f32, tag="mm")
                for k in range(4):
                    nc.tensor.matmul(
                        acc[:], wcvcq_sb[:, 128 * k:128 * (k + 1)],
                        cfr_sb[k][:, 512 * half:512 * (half + 1)],
                        start=(k == 0), stop=(k == 3))
                nc.vector.tensor_copy(
                    ocf_sb[:, 512 * half:512 * (half + 1)], acc[:])
                if half == 1:
                    nc.sync.dma_start(out_cf[:, :], ocf_sb[:])

            def proj_round(jlo, u, last=False):
                """Proj partial for head-pairs (jlo, jlo+1), tiles 2u,2u+1."""
                for t in (2 * u, 2 * u + 1):
                    acc = psT.tile([128, 512], f32, tag="mm")
                    for jj in (jlo, jlo + 1):
                        nc.tensor.matmul(acc[:],
                                         packed[jj][:, 128 * t:128 * (t + 1)],
                                         wproj_sb[:, 512 * jj:512 * (jj + 1)],
                                         start=(jj == jlo), stop=(jj != jlo))
                    if not last:
                        nc.vector.tensor_copy(s_acc[t], acc[:])
                    else:
                        nc.vector.tensor_add(s_sb[t], s_acc[t], acc[:])
                if last:
                    qs = (nc.sync, nc.gpsimd, nc.scalar, nc.gpsimd)
                    for g in range(2):
                        for sh in range(2):
                            qs[2 * g + sh].dma_start(
                                s2d[u][64 * sh:64 * sh + 64,
                                       512 * g:512 * g + 512],
                                s_sb[2 * u + sh][g:128:2, :])

            filler = {}
            filler[0, 0] = late_dmas
            for kt in range(1, 7):  # v groups 2..7, all before their PVs
                filler[0, kt] = lambda kt=kt: v_group(psT, kt + 1)
            for hh in range(1, 4):  # k/q groups for pairs 1..3
                for sl_i in range(4):
                    filler[hh, 2 * sl_i + 1] = (
                        lambda hh=hh, sl_i=sl_i: kq_part(psT, hh, sl_i))
            for u in range(4):  # proj round for pairs 0+1
                filler[4, 2 * u + 1] = lambda u=u: proj_round(0, u)
            filler[5, 1] = lambda: scq_part(0)
            filler[5, 3] = lambda: scq_part(1)
            filler[5, 5] = lambda: ocf_part(0)
            filler[5, 7] = lambda: ocf_part(1)

            # head 6 (even parity: short norm chain) goes LAST so the final
            # normalization before the proj tail is the cheap direct-mul one.
            # The final PV pair + norm of head h are deferred until after the
            # NEXT head's first scores are issued, keeping ACT fed across
            # head boundaries.
            def pv_pair(o_ps, h, ktp):
                vv = v_sb[ktp][:].rearrange("p (i c) -> p i c", i=2)
                pp = ptp[ktp][:].rearrange("p (i c) -> p i c", i=2)
                for i in range(2):
                    for qc in range(2):
                        nc.tensor.matmul(
                            o_ps[:, 512 * qc:512 * (qc + 1)],
                            vv[:, i, 128 * h:128 * (h + 1)],
                            pp[:, i, 512 * qc:512 * (qc + 1)],
                            start=(ktp == 0 and i == 0),
                            stop=(ktp == 3 and i == 1))

            def norm_chain(o_ps, h, hi, split=False):
                par, j = hi % 2, h // 2
                rows = slice(64 * (h % 2), 64 * (h % 2) + 64)
                if split:
                    # final head: pipeline the chain in qc-halves so the
                    # Pool broadcast overlaps the DVE recip of the other half
                    for c in (slice(0, 512), slice(512, 1024)):
                        nc.vector.tensor_copy(zraw[par][0:1, c],
                                              o_ps[64:65, c])
                        nc.vector.reciprocal_approx_accurate(
                            zinv[par][0:1, c], zraw[par][0:1, c], zs2[0:1, c])
                        nc.gpsimd.partition_broadcast(zbc[par][:, c],
                                                      zinv[par][0:1, c])
                        nc.vector.tensor_mul(packed[j][rows, c],
                                             o_ps[0:64, c], zbc[par][rows, c])
                    return
                nc.vector.tensor_copy(zraw[par][0:1, :], o_ps[64:65, :])
                nc.vector.reciprocal_approx_accurate(
                    zinv[par][0:1, :], zraw[par][0:1, :], zs2[0:1, :])
                nc.gpsimd.partition_broadcast(zbc[par][:], zinv[par][0:1, :])
                if h % 2 == 0:
                    nc.vector.tensor_mul(packed[j][rows, :], o_ps[0:64, :],
                                         zbc[par][rows, :])
                else:
                    nc.vector.tensor_copy(packed[j][rows, :], o_ps[0:64, :])
                    nc.gpsimd.tensor_mul(packed[j][rows, :],
                                         packed[j][rows, :],
                                         zbc[par][rows, :])

            prev_tail = None
            for hi, h in enumerate([0, 1, 2, 3, 4, 5, 7, 6]):
                j, row = h // 2, 64 * (h % 2)
                o_ps = psO.tile([128, S], f32, tag="o")
                pend = []  # PV runs one kt-pair behind to hide slot waits
                for kt in range(8):
                    sc = psS.tile([128, S], f32, tag="sc")
                    for qc in range(2):
                        nc.tensor.matmul(
                            sc[:, 512 * qc:512 * (qc + 1)],
                            kT[j][row:row + 64, 128 * kt:128 * (kt + 1)],
                            qT[j][row:row + 64, 512 * qc:512 * (qc + 1)],
                            start=True, stop=True)
                    nc.scalar.activation(
                        ptp[kt // 2][:, 1024 * (kt % 2):1024 * (kt % 2) + 1024],
                        sc[:], EXP, scale=SCALE)
                    if kt == 0 and prev_tail is not None:
                        prev_tail()
                    if kt % 2 == 1:
                        pend.append(kt // 2)
                    if len(pend) > 1:
                        pv_pair(o_ps, h, pend.pop(0))
                    f = filler.get((hi, kt))
                    if f is not None:
                        f()

                def mk_tail(o_ps=o_ps, h=h, hi=hi, ktp=pend[0]):
                    def tail():
                        pv_pair(o_ps, h, ktp)
                        norm_chain(o_ps, h, hi, split=(hi == 7))
                    return tail
                prev_tail = mk_tail()
            prev_tail()
            attn_ctx.close()  # release the 6 attention PSUM banks

            # ---- tail: proj round for pairs 2+3 -> s2d -> conv.  The conv
            # runs 8 accumulation groups in parallel banks so each s2d[r]
            # is consumed the moment its re-pair DMA lands. ----
            with tc.tile_pool(name="psF", bufs=1, space="PSUM") as psF:
                accs = []
                for g in range(8):
                    if g < 6:
                        accs.append(psF.tile([128, 512], f32, tag=f"cv{g}",
                                             name=f"cv{g}"))
                    else:
                        accs.append(psT.tile([128, 512], f32, tag="mm",
                                             name=f"cv{g}"))

                def conv_head(g):
                    oc, half = g // 2, g % 2
                    nc.tensor.matmul(
                        accs[g][:],
                        bias_sb[0:1, 512 + 128 * oc:512 + 128 * (oc + 1)],
                        ones[0:1, :], start=True, stop=False)
                    nc.tensor.matmul(
                        accs[g][:], wcvsq_sb[:, 128 * oc:128 * (oc + 1)],
                        s2d_cq[:, 512 * half:512 * (half + 1)],
                        start=False, stop=False)

                for g in range(6):
                    conv_head(g)
                for u in range(4):
                    proj_round(2, u, last=True)
                for g in (6, 7):
                    conv_head(g)
                for r in range(4):
                    for g in range(8):
                        oc, half = g // 2, g % 2
                        nc.tensor.matmul(
                            accs[g][:],
                            wcvs_sb[:, 512 * r + 128 * oc:
                                    512 * r + 128 * (oc + 1)],
                            s2d[r][:, 512 * half:512 * (half + 1)],
                            start=False, stop=(r == 3))
                for g in range(8):
                    oc, half = g // 2, g % 2
                    eng = nc.vector if g % 2 == 0 else nc.scalar
                    if eng is nc.scalar:
                        eng.copy(outp[oc][:, 512 * half:512 * (half + 1)],
                                 accs[g][:])
                    else:
                        eng.tensor_copy(
                            outp[oc][:, 512 * half:512 * (half + 1)],
                            accs[g][:])
                    eng2 = nc.sync if g % 2 == 0 else nc.gpsimd
                    eng2.dma_start(
                        out_p[128 * oc:128 * (oc + 1),
                              512 * half:512 * (half + 1)],
                        outp[oc][:, 512 * half:512 * (half + 1)])

    nc.compile()
    _CACHE["nc"] = nc
    return nc


def _shard_inputs(content_feat, components, pos_emb, Wq, Wkv, Wproj, bproj,
                  Wconv, bconv):
    import ml_dtypes
    bf = ml_dtypes.bfloat16
    f = np.float32
    posT = pos_emb.reshape(S, C).T.astype(f)
    wconvT = Wconv.T.astype(f)                        # [2C, C]
    wk_img = _img(np.ascontiguousarray(Wkv[:, :C]), C)
    wv_img = _img(np.ascontiguousarray(Wkv[:, C:]), C)
    wq_img = _img(Wq, C)
    wkq_img = np.concatenate([wk_img, wq_img], axis=1)
    wproj_img = _img(Wproj, C)
    wcvs_img = _img(np.ascontiguousarray(wconvT[:C]), C)
    bias2 = np.ascontiguousarray(
        np.concatenate([bproj, bconv / 4]).reshape(1, 1024)).astype(bf)
    in_maps = []
    for core in range(N_CORES):
        b, n = core // 4, core % 4
        sl = slice(128 * n, 128 * (n + 1))
        tq = slice(256 * n, 256 * (n + 1))
        ctok_f = content_feat[b].reshape(S, C).T + posT     # [C, S]
        ctmp_f = components[n, b].reshape(S, C).T + posT
        in_maps.append({
            "ctm": _img(ctmp_f, S),
            "ctk": _img(ctok_f, S),
            "cfr": _img(content_feat[b].reshape(C, S), S),
            "wkq": wkq_img,
            "wv": wv_img,
            "wproj": wproj_img,
            "wcvs": wcvs_img,
            "wcvcq": _img(np.ascontiguousarray(wconvT[C:, sl]), 128),
            "wcvsq": np.ascontiguousarray(wconvT[sl]).astype(bf),
            "ctkq": _img(np.ascontiguousarray(ctok_f[:, tq]), 256),
            "bias2": bias2,
        })
    return in_maps


def _run(trace=False, **inputs):
    from concourse.bass_utils import run_bass_kernel_spmd

    nc = _build()
    in_maps = _shard_inputs(**inputs)
    res = run_bass_kernel_spmd(nc, in_maps, list(range(N_CORES)), trace=trace)
    full = np.empty((B, C, S), dtype=np.float32)
    for b in range(B):
        acc = sum(res.results[4 * b + n]["out_p"].astype(np.float32)
                  for n in range(4))
        for n in range(4):
            acc[128 * n:128 * (n + 1)] += \
                res.results[4 * b + n]["out_cf"].astype(np.float32)
        full[b] = acc
    return full.reshape(B, C, H, W).astype(np.float32), res


def kernel(**inputs):
    out, _ = _run(trace=False, **inputs)
    return out


# revision 49
# speedup vs baseline: 1.0675x; 1.0675x over previous
"""Trainium2 Bass kernel for nn_Attention_54391465836966.

The reference's .reshape calls are RAW byte reinterpretations: token matrix
T = content_feat[b] bytes viewed [S, C] (not a transpose), and s (token-major
[S, C]) is viewed [C, S] before the 1x1 conv.  The host passes every input
pre-arranged into its exact SBUF image (one [128, X] contiguous DMA each, in
bf16), with the token views pre-transposed to channel-major, so the device
does no PE transposes; the s view is realized with SBUF->SBUF DMAs that
re-pair token rows (s2d[r] = tokens (2r, 2r+1) concatenated).

Per core (b = core//4, n = core%4), channel-major [C, S] throughout:
  ctok = cfT + posT ; ctmp = compT + posT
  qT = Wq^T ctok ; kT = Wkv[:, :C]^T ctmp ; v = ctmp^T Wkv[:, C:]
  per head h: P = exp(scale k_h^T q); o_h = (v_h^T P) / Z   (Z via ones col)
  s_tok = packed^T Wproj                                     (token-major)
  const (token-quarter n, full scale): s_cq = ctokQ^T Wproj + bproj
  out_p = WconvT[:C]^T s2d + WconvT[quarter]^T s2d_cq + bconv/4
  out_cf = WconvT[C:, out-quarter]^T cf_raw                  (host-placed)
Host sums the 4 component partials per batch and places out_cf quarter rows.
The affine const terms are distributed so no gated-zero work exists.

Dtypes: bf16 throughout (PE rate = fp32r, half the DMA/SBUF traffic; DVE
adds get the 2x mode); PSUM and the softmax-normalization scratch stay f32.

Schedule: attention is ACT(exp)-bound at ~1.2us/kt, so only v and the
(kT, qT) pair for head pair 0 are computed up front; everything else that
does not gate the exp stream — the remaining k/q groups, the const paths,
and the per-head-pair proj partial sums — is emitted INTO the head loop to
fill PE slack under the exps.  Only the last pair's proj round, the s2d
re-pair, and the conv remain in the tail.  The z-scratch is double-buffered
by head parity; head 6 (cheap even-parity norm) is processed last.
"""
import sys

sys.path.insert(0, "/opt/trn_rl_repo")

import numpy as np

N_CORES = 8
B, C, H, W = 2, 512, 32, 32
S = H * W  # 1024
NH, HD = 8, 64
SCALE = HD ** -0.5

_CACHE = {}


def _img(x, cols):
    """[512, cols] matrix -> its [128, 4*cols] SBUF image (4 row-blocks
    side by side), in bf16."""
    import ml_dtypes
    return np.ascontiguousarray(
        x.reshape(4, 128, cols).transpose(1, 0, 2).reshape(128, 4 * cols)
    ).astype(ml_dtypes.bfloat16)


def _build():
    if "nc" in _CACHE:
        return _CACHE["nc"]
    from contextlib import ExitStack

    import concourse.bacc as bacc
    import concourse.mybir as mybir
    import concourse.tile as tile

    f32 = mybir.dt.float32
    bf16 = mybir.dt.bfloat16
    fp8v = mybir.dt.float8e4
    fp8p = mybir.dt.float8e5
    EXP = mybir.ActivationFunctionType.Exp
    DR = mybir.MatmulPerfMode.DoubleRow

    nc = bacc.Bacc("TRN2", target_bir_lowering=False, debug=False,
                   num_devices=N_CORES)

    din = lambda n, s: nc.dram_tensor(n, s, mybir.dt.bfloat16,
                                      kind="ExternalInput").ap()
    ctm_d = din("ctm", [128, 4096])      # (compT + posT) image
    ctk_d = din("ctk", [128, 4096])      # (cfT + posT) image
    cfr_d = din("cfr", [128, 4096])      # raw content_feat[b] image
    wkq_d = din("wkq", [128, 4096])      # [Wkv[:, :C] | Wq] images
    wv_d = din("wv", [128, 2048])        # Wkv[:, C:] image
    wproj_d = din("wproj", [128, 2048])  # Wproj image
    wcvs_d = din("wcvs", [128, 2048])    # WconvT[:C] image
    wcvcq_d = din("wcvcq", [128, 512])   # WconvT[C:, out-quarter] image
    wcvsq_d = din("wcvsq", [128, 512])   # WconvT[128n:128(n+1), :]
    ctkq_d = din("ctkq", [128, 1024])    # ctok[:, token-quarter] image
    bias_d = din("bias2", [1, 1024])     # [bproj, bconv/4]
    out_p = nc.dram_tensor("out_p", [C, S], bf16, kind="ExternalOutput").ap()
    out_cf = nc.dram_tensor("out_cf", [128, S], bf16,
                            kind="ExternalOutput").ap()

    with tile.TileContext(nc) as tc, ExitStack() as ctx:
        main = ctx.enter_context(tc.tile_pool(name="main", bufs=1))

        ones = main.tile([1, 512], bf16, tag="ones")
        nc.gpsimd.memset(ones[:], 1.0)

        # ---- front-critical DMAs, split across the SP and Pool queues so
        # issue overhead (~1.5us/DMA on SP) parallelizes ----
        ctm_big = main.tile([128, 4096], bf16, tag="ctmB")
        ctk_big = main.tile([128, 4096], bf16, tag="ctkB")
        ctmp = [ctm_big[:, 1024 * j:1024 * (j + 1)] for j in range(4)]
        ctok = [ctk_big[:, 1024 * j:1024 * (j + 1)] for j in range(4)]
        wv_sb = main.tile([128, 2048], bf16, tag="wv")
        wkq_big = main.tile([128, 4096], bf16, tag="wkqB")
        wk_sb = wkq_big[:, 0:2048]
        wq_sb = wkq_big[:, 2048:4096]
        nc.sync.dma_start(ctm_big[:], ctm_d[:])
        nc.gpsimd.dma_start(ctk_big[:], ctk_d[:])
        nc.sync.dma_start(wkq_big[:], wkq_d[:])
        nc.gpsimd.dma_start(wv_sb[:], wv_d[:])

        # ---- late weights / const-path inputs (Pool queue) ----
        cfr_big = main.tile([128, 4096], bf16, tag="cfrB")
        cfr_sb = [cfr_big[:, 1024 * j:1024 * (j + 1)] for j in range(4)]
        wproj_sb = main.tile([128, 2048], bf16, tag="wp")
        wcvs_sb = main.tile([128, 2048], bf16, tag="wcs")
        wcvcq_sb = main.tile([128, 512], bf16, tag="wcc")
        wcvsq_sb = main.tile([128, 512], bf16, tag="wcsq")
        ctokq = main.tile([128, 1024], bf16, tag="ctokq")
        bias_sb = main.tile([1, 1024], bf16, tag="bias")
        def late_dmas():
            nc.gpsimd.dma_start(wproj_sb[:], wproj_d[:])
            nc.gpsimd.dma_start(cfr_big[:], cfr_d[:])
            nc.gpsimd.dma_start(wcvcq_sb[:], wcvcq_d[:])
            nc.gpsimd.dma_start(ctokq[:], ctkq_d[:])
            nc.gpsimd.dma_start(bias_sb[:], bias_d[:])
            nc.gpsimd.dma_start(wcvs_sb[:], wcvs_d[:])
            nc.gpsimd.dma_start(wcvsq_sb[:], wcvsq_d[:])

        # norm scratch (x2 by parity, shared zs2) + late tiles
        zraw = [main.tile([1, S], f32, tag=f"zraw{i}", name=f"zraw{i}")
                for i in range(2)]
        zs2 = main.tile([1, S], f32, tag="zs2")
        zinv = [main.tile([1, S], f32, tag=f"zinv{i}", name=f"zinv{i}")
                for i in range(2)]
        zbc = [main.tile([128, S], f32, tag=f"zbc{i}", name=f"zbc{i}")
               for i in range(2)]
        ocf_sb = main.tile([128, S], bf16, tag="ctkB", name="ocf")
        packed = [main.tile([128, S], bf16, tag=f"pk{j}", name=f"pk{j}")
                  for j in range(4)]
        outp = [main.tile([128, S], bf16, tag=f"op{j}", name=f"op{j}")
                for j in range(4)]
        kT = [main.tile([128, S], bf16, tag=f"kT{j}", name=f"kT{j}")
              for j in range(4)]
        qT = [main.tile([128, S], bf16, tag=f"qT{j}", name=f"qT{j}")
              for j in range(4)]
        # v pairs: [p, (i, 128h + [64 d | Z | 63 pad])], pads/Z = 1.0
        v_sb = [main.tile([128, 2048], fp8v, tag=f"v{t}", name=f"v{t}")
                for t in range(4)]
        bneg = main.tile([128, 1], f32, tag="bneg")
        # proj partial accumulators (f32) ride the dead pos slot; final s
        # tiles ride cmp; the re-paired s2d view rides cfr
        s_acc_big = main.tile([128, 4096], f32, tag="ctmB", name="sacc")
        s_acc = [s_acc_big[:, 512 * i:512 * (i + 1)] for i in range(8)]
        s_sb_big = main.tile([128, 4096], bf16, tag="wkqB", name="ssb")
        s_sb = [s_sb_big[:, 512 * i:512 * (i + 1)] for i in range(8)]
        s2d_big = main.tile([128, 4096], bf16, tag="cfrB", name="s2db")
        s2d = [s2d_big[:, 1024 * i:1024 * (i + 1)] for i in range(4)]

        def kq_part(psum, j, part):
            """One of four (kT/qT, qc) projection groups for head-pair j."""
            kind, qc = part % 2, part // 2
            w, act, dst = ((wk_sb, ctmp, kT) if kind == 0
                           else (wq_sb, ctok, qT))
            acc = psum.tile([128, 512], f32, tag="mm")
            for k in range(4):
                nc.tensor.matmul(
                    acc[:],
                    w[:, 512 * k + 128 * j:512 * k + 128 * (j + 1)],
                    act[k][:, 512 * qc:512 * (qc + 1)],
                    start=(k == 0), stop=(k == 3))
            nc.vector.tensor_copy(dst[j][:, 512 * qc:512 * (qc + 1)], acc[:])

        def v_group(psum, kt):
            acc = psum.tile([128, 512], f32, tag="mm")
            for k in range(4):
                nc.tensor.matmul(acc[:],
                                 ctmp[k][:, 128 * kt:128 * (kt + 1)],
                                 wv_sb[:, 512 * k:512 * (k + 1)],
                                 start=(k == 0), stop=(k == 3))
            dst = v_sb[kt // 2][:, 1024 * (kt % 2):1024 * (kt % 2) + 1024]
            nc.scalar.copy(
                dst.rearrange("p (m c) -> p m c", m=8)[:, :, 0:64],
                acc[:].rearrange("p (m c) -> p m c", m=8))

        with tc.tile_pool(name="psA", bufs=2, space="PSUM") as ps:
            for t in range(4):
                nc.gpsimd.memset(v_sb[t][:], 1.0)
            for part in range(4):
                kq_part(ps, 0, part)

        # ---- attention, with remaining work streamed into PE slack ----
        with tc.tile_pool(name="psT", bufs=2, space="PSUM") as psT:
            attn_ctx = ExitStack()
            psS = attn_ctx.enter_context(
                tc.tile_pool(name="psS", bufs=2, space="PSUM"))
            psO = attn_ctx.enter_context(
                tc.tile_pool(name="psO", bufs=1, space="PSUM"))
            ptp = [main.tile([128, 2048], fp8p, tag=f"pt{t}", name=f"pt{t}")
                   for t in range(4)]
            nc.gpsimd.memset(bneg[:], -6.0)
            s_cq = [main.tile([128, 512], bf16, tag=f"scq{i}", name=f"scq{i}")
                    for i in range(2)]
            s2d_cq = main.tile([128, S], bf16, tag="s2dcq")

            def scq_part(i):
                acc = psT.tile([128, 512], f32, tag="mm")
                nc.tensor.matmul(acc[:], ones[0:1, 0:128],
                                 bias_sb[0:1, 0:512], start=True, stop=False)
                for a in range(4):
                    nc.tensor.matmul(
                        acc[:],
                        ctokq[:, 256 * a + 128 * i:256 * a + 128 * (i + 1)],
                        wproj_sb[:, 512 * a:512 * (a + 1)],
                        start=False, stop=(a == 3))
                nc.vector.tensor_copy(s_cq[i][:], acc[:])
                if i == 1:
                    for g in range(2):
                        for sh in range(2):
                            eng = nc.sync if g == 0 else nc.gpsimd
                            eng.dma_start(
                                s2d_cq[64 * sh:64 * sh + 64,
                                       512 * g:512 * g + 512],
                                s_cq[sh][g:128:2, :])

            def ocf_part(half):
                acc = psT.tile([128, 512], f32, tag="mm")
                for k in range(4):
                    nc.tensor.matmul(
                        acc[:], wcvcq_sb[:, 128 * k:128 * (k + 1)],
                        cfr_sb[k][:, 512 * half:512 * (half + 1)],
                        start=(k == 0), stop=(k == 3))
                nc.vector.tensor_copy(
                    ocf_sb[:, 512 * half:512 * (half + 1)], acc[:])
                if half == 1:
                    nc.sync.dma_start(out_cf[:, :], ocf_sb[:])

            def proj_round(jlo, u, last=False):
                """Proj partial for head-pairs (jlo, jlo+1), tiles 2u,2u+1."""
                for t in (2 * u, 2 * u + 1):
                    acc = psT.tile([128, 512], f32, tag="mm")
                    for jj in (jlo, jlo + 1):
                        nc.tensor.matmul(acc[:],
                                         packed[jj][:, 128 * t:128 * (t + 1)],
                                         wproj_sb[:, 512 * jj:512 * (jj + 1)],
                                         start=(jj == jlo), stop=(jj != jlo))
                    if not last:
                        nc.vector.tensor_copy(s_acc[t], acc[:])
                    else:
                        nc.vector.tensor_add(s_sb[t], s_acc[t], acc[:])
                if last:
                    qs = (nc.sync, nc.gpsimd, nc.scalar, nc.gpsimd)
                    for g in range(2):
                        for sh in range(2):
                            qs[2 * g + sh].dma_start(
                                s2d[u][64 * sh:64 * sh + 64,
                                       512 * g:512 * g + 512],
                                s_sb[2 * u + sh][g:128:2, :])

            filler = {}
            filler[0, 0] = late_dmas
            for kt in range(1, 8):  # v groups 0..6, each before its PV
                filler[0, kt] = lambda kt=kt: v_group(psT, kt - 1)
            for hh in range(1, 4):  # k/q groups for pairs 1..3
                for sl_i in range(4):
                    filler[hh, 2 * sl_i + 1] = (
                        lambda hh=hh, sl_i=sl_i: kq_part(psT, hh, sl_i))
            for u in range(4):  # proj round for pairs 0+1
                filler[4, 2 * u + 1] = lambda u=u: proj_round(0, u)
            filler[5, 1] = lambda: scq_part(0)
            filler[5, 3] = lambda: scq_part(1)
            filler[5, 5] = lambda: ocf_part(0)
            filler[5, 7] = lambda: ocf_part(1)

            # head 6 (even parity: short norm chain) goes LAST so the final
            # normalization before the proj tail is the cheap direct-mul one.
            # The final PV pair + norm of head h are deferred until after the
            # NEXT head's first scores are issued, keeping ACT fed across
            # head boundaries.
            def pv_pair(o_ps, h, ktp):
                vv = v_sb[ktp][:].rearrange("p (i c) -> p i c", i=2)
                pp = ptp[ktp][:].rearrange("p (i c) -> p i c", i=2)
                for qc in range(2):
                    nc.tensor.matmul(
                        o_ps[:, 512 * qc:512 * (qc + 1)],
                        vv[:, :, 128 * h:128 * (h + 1)],
                        pp[:, :, 512 * qc:512 * (qc + 1)],
                        start=(ktp == 0), stop=(ktp == 3), perf_mode=DR)

            def norm_chain(o_ps, h, hi, split=False):
                par, j = hi % 2, h // 2
                rows = slice(64 * (h % 2), 64 * (h % 2) + 64)
                if split:
                    # final head: pipeline the chain in qc-halves so the
                    # Pool broadcast overlaps the DVE recip of the other half
                    for c in (slice(0, 512), slice(512, 1024)):
                        nc.vector.tensor_copy(zraw[par][0:1, c],
                                              o_ps[64:65, c])
                        nc.vector.reciprocal_approx_accurate(
                            zinv[par][0:1, c], zraw[par][0:1, c], zs2[0:1, c])
                        nc.gpsimd.partition_broadcast(zbc[par][:, c],
                                                      zinv[par][0:1, c])
                        nc.vector.tensor_mul(packed[j][rows, c],
                                             o_ps[0:64, c], zbc[par][rows, c])
                    return
                nc.vector.tensor_copy(zraw[par][0:1, :], o_ps[64:65, :])
                nc.vector.reciprocal_approx_accurate(
                    zinv[par][0:1, :], zraw[par][0:1, :], zs2[0:1, :])
                nc.gpsimd.partition_broadcast(zbc[par][:], zinv[par][0:1, :])
                if h % 2 == 0:
                    nc.vector.tensor_mul(packed[j][rows, :], o_ps[0:64, :],
                                         zbc[par][rows, :])
                else:
                    nc.vector.tensor_copy(packed[j][rows, :], o_ps[0:64, :])
                    nc.gpsimd.tensor_mul(packed[j][rows, :],
                                         packed[j][rows, :],
                                         zbc[par][rows, :])

            prev_tail = None
            for hi, h in enumerate([0, 1, 2, 3, 4, 5, 7, 6]):
                j, row = h // 2, 64 * (h % 2)
                o_ps = psO.tile([128, S], f32, tag="o")
                pend = []  # PV runs one kt-pair behind to hide slot waits
                for kt in range(8):
                    sc = psS.tile([128, S], f32, tag="sc")
                    for qc in range(2):
                        nc.tensor.matmul(
                            sc[:, 512 * qc:512 * (qc + 1)],
                            kT[j][row:row + 64, 128 * kt:128 * (kt + 1)],
                            qT[j][row:row + 64, 512 * qc:512 * (qc + 1)],
                            start=True, stop=True)
                    nc.scalar.activation(
                        ptp[kt // 2][:, 1024 * (kt % 2):1024 * (kt % 2) + 1024],
                        sc[:], EXP, scale=SCALE, bias=bneg[:, 0:1])
                    if kt == 0 and prev_tail is not None:
                        prev_tail()
                    if kt % 2 == 1:
                        pend.append(kt // 2)
                    if len(pend) > 1:
                        pv_pair(o_ps, h, pend.pop(0))
                    f = filler.get((hi, kt))
                    if f is not None:
                        f()

                def mk_tail(o_ps=o_ps, h=h, hi=hi, ktp=pend[0]):
                    def tail():
                        if hi == 0:  # last v group rides ahead of its PV
                            v_group(psT, 7)
                        pv_pair(o_ps, h, ktp)
                        norm_chain(o_ps, h, hi, split=(hi == 7))
                    return tail
                prev_tail = mk_tail()
            prev_tail()
            attn_ctx.close()  # release the 6 attention PSUM banks

            # ---- tail: proj round for pairs 2+3 -> s2d -> conv.  The conv
            # runs 8 accumulation groups in parallel banks so each s2d[r]
            # is consumed the moment its re-pair DMA lands. ----
            with tc.tile_pool(name="psF", bufs=1, space="PSUM") as psF:
                accs = []
                for g in range(8):
                    if g < 6:
                        accs.append(psF.tile([128, 512], f32, tag=f"cv{g}",
                                             name=f"cv{g}"))
                    else:
                        accs.append(psT.tile([128, 512], f32, tag="mm",
                                             name=f"cv{g}"))

                def conv_head(g):
                    oc, half = g // 2, g % 2
                    nc.tensor.matmul(
                        accs[g][:],
                        bias_sb[0:1, 512 + 128 * oc:512 + 128 * (oc + 1)],
                        ones[0:1, :], start=True, stop=False)
                    nc.tensor.matmul(
                        accs[g][:], wcvsq_sb[:, 128 * oc:128 * (oc + 1)],
                        s2d_cq[:, 512 * half:512 * (half + 1)],
                        start=False, stop=False)

                for g in range(6):
                    conv_head(g)
                for u in range(4):
                    proj_round(2, u, last=True)
                for g in (6, 7):
                    conv_head(g)
                for r in range(4):
                    for g in range(8):
                        oc, half = g // 2, g % 2
                        nc.tensor.matmul(
                            accs[g][:],
                            wcvs_sb[:, 512 * r + 128 * oc:
                                    512 * r + 128 * (oc + 1)],
                            s2d[r][:, 512 * half:512 * (half + 1)],
                            start=False, stop=(r == 3))
                for g in range(8):
                    oc, half = g // 2, g % 2
                    eng = nc.vector if g % 2 == 0 else nc.scalar
                    if eng is nc.scalar:
                        eng.copy(outp[oc][:, 512 * half:512 * (half + 1)],
                                 accs[g][:])
                    else:
                        eng.tensor_copy(
                            outp[oc][:, 512 * half:512 * (half + 1)],
                            accs[g][:])
                    eng2 = nc.sync if g % 2 == 0 else nc.gpsimd
                    eng2.dma_start(
                        out_p[128 * oc:128 * (oc + 1),
                              512 * half:512 * (half + 1)],
                        outp[oc][:, 512 * half:512 * (half + 1)])

    nc.compile()
    _CACHE["nc"] = nc
    return nc


def _shard_inputs(content_feat, components, pos_emb, Wq, Wkv, Wproj, bproj,
                  Wconv, bconv):
    import ml_dtypes
    bf = ml_dtypes.bfloat16
    f = np.float32
    posT = pos_emb.reshape(S, C).T.astype(f)
    wconvT = Wconv.T.astype(f)                        # [2C, C]
    wk_img = _img(np.ascontiguousarray(Wkv[:, :C]), C)
    wv_img = _img(np.ascontiguousarray(Wkv[:, C:]), C)
    wq_img = _img(Wq, C)
    wkq_img = np.concatenate([wk_img, wq_img], axis=1)
    wproj_img = _img(Wproj, C)
    wcvs_img = _img(np.ascontiguousarray(wconvT[:C]), C)
    bias2 = np.ascontiguousarray(
        np.concatenate([bproj, bconv / 4]).reshape(1, 1024)).astype(bf)
    in_maps = []
    for core in range(N_CORES):
        b, n = core // 4, core % 4
        sl = slice(128 * n, 128 * (n + 1))
        tq = slice(256 * n, 256 * (n + 1))
        ctok_f = content_feat[b].reshape(S, C).T + posT     # [C, S]
        ctmp_f = components[n, b].reshape(S, C).T + posT
        in_maps.append({
            "ctm": _img(ctmp_f, S),
            "ctk": _img(ctok_f, S),
            "cfr": _img(content_feat[b].reshape(C, S), S),
            "wkq": wkq_img,
            "wv": wv_img,
            "wproj": wproj_img,
            "wcvs": wcvs_img,
            "wcvcq": _img(np.ascontiguousarray(wconvT[C:, sl]), 128),
            "wcvsq": np.ascontiguousarray(wconvT[sl]).astype(bf),
            "ctkq": _img(np.ascontiguousarray(ctok_f[:, tq]), 256),
            "bias2": bias2,
        })
    return in_maps


def _run(trace=False, **inputs):
    from concourse.bass_utils import run_bass_kernel_spmd

    nc = _build()
    in_maps = _shard_inputs(**inputs)
    res = run_bass_kernel_spmd(nc, in_maps, list(range(N_CORES)), trace=trace)
    full = np.empty((B, C, S), dtype=np.float32)
    for b in range(B):
        acc = sum(res.results[4 * b + n]["out_p"].astype(np.float32)
                  for n in range(4))
        for n in range(4):
            acc[128 * n:128 * (n + 1)] += \
                res.results[4 * b + n]["out_cf"].astype(np.float32)
        full[b] = acc
    return full.reshape(B, C, H, W).astype(np.float32), res


def kernel(**inputs):
    out, _ = _run(trace=False, **inputs)
    return out
